# revision 26
# baseline (speedup 1.0000x reference)
"""Trainium2 Bass kernel for nn_BasicBlock_1w4a_LUT (binarized 3x3 conv + LUT bucketize).

Data-parallel over batch: 8 NeuronCores x 4 images each; no cross-core
communication. Full inputs in, full output out; shard/unshard on the host.

Host prep:
  - Binarize the weights exactly as the reference does; the result is
    sign(bw)*sw with sw>0 per out-channel. sw is folded into the LUT
    thresholds so the device weights are exactly +/-1 (exact in fp16).
  - x cast to a single fp16 copy (the 2e-2 tolerance allows the ~5e-4
    bucket-flip rate this costs) and split into two overlapping horizontal
    halves of the zero-padded 114x114 image: partitions 0-63 hold padded
    rows 0-58, partitions 64-127 hold padded rows 55-113 (3-row halo).
    Each pixel is stored once (plus the tiny halo), so input DMA is half
    of a full duplicated-slab scheme.
  - Per-channel affine z = y*s + b chosen so bucketize thresholds map to
    tau3 -> 0 and tau5 -> 1 (frees DVE scalar slots; the DVE op set below
    only has 3 per-partition scalar slots per instruction).

Device, per image:
  - The 3x3/pad-1 conv runs as 9 accumulated K=64 fp16 matmuls per
    448-pixel chunk (4 output rows), with FOUR chunks streaming through
    the PE concurrently via 2x2 tile_position quadrants: row-groups 0/64
    process top/bottom image halves, col-groups 0/64 process adjacent
    chunks. 4 x (64x64) tiles = all 16384 MACs/cycle busy -> the fp16
    MAC-floor (~47us for 4 images).
  - ScalarE drains each PSUM bank with the per-channel affine to an fp16
    per-image z slab; two custom DVE ops then bucketize half an image per
    instruction (sum of 7 [z > tau_k] compares in 2 passes, 5+8 ALU
    stages), writing u8.
  - A few junk matmuls at kernel start warm the PE HAM clock gate
    (1.2 -> 2.4 GHz) while the first input DMA pieces land; input/output
    DMAs are split so compute starts/finishes without waiting for whole
    images.
"""

import numpy as np

# ---- problem constants (hardcoded per contract) ----
B, Cin, Cout, H, W = 32, 64, 64, 112, 112
NCORES = 8
BPC = B // NCORES          # images per core
HP = H + 2                 # 114 padded rows
WPAD = W + 2               # 114 padded cols
HALF = H * W // 2          # 6272 output pixels per column-tile half
NCH = 4 * W                # 448 pixels per chunk = 4 image rows (PSUM <= 2KB)
NPAIR = H // 8             # 14 chunk pairs per image (7 per half)
HROWS = 59                 # padded rows stored per partition half
SLABF = HROWS * WPAD       # 6726 flat elements per half-slab
NTAPS = 9
NWARM = 8                  # PE warm-up matmuls

_built = []
last_results = None


def _register_dve_ops():
    from concourse.dve_spec import (
        Spec, Src0, Src1, C0, C1, C3, One, Zero, lower,
        _spill_c3_to_src1, _has_src1,
    )
    import concourse.dve_ops as dve_ops
    from concourse.dve_ops import DveOp
    from concourse.dve_uop import DveOpSpec

    def register_op(name, spec):
        if name in dve_ops._SUB_OPCODE_FOR_NAME:
            for op in dve_ops.OPS:
                if op.name == name:
                    return op
            raise RuntimeError(name)
        row = max(dve_ops._SUB_OPCODE_FOR_NAME.values()) + 1
        assert row < 0x20, "custom-DVE opcode rows exhausted"
        shas = {}
        for ver in ("v3", "v4"):
            s = DveOpSpec(name=name, opcode=row, uops=lower(spec, ver=ver),
                          rd1_en=_has_src1(spec))
            shas[ver] = s.sha(ver)
        op = DveOp(name, spec, subdim=False, uops_sha=shas)
        dve_ops.OPS.append(op)
        dve_ops.CUSTOM_DVE_SPECS[name] = spec
        dve_ops._SUB_OPCODE_FOR_NAME[name] = row
        return op

    # u = (z>tau0) + (z>tau1) + (z>tau2);  tau2 rides C3 (spilled to in1
    # [P,1]).  3 cmp + 2 add = 5 ALU stages.
    bucket3 = register_op(
        "BUCKET3_ANT",
        Spec(
            body=_spill_c3_to_src1(((Src0 > C0) + (Src0 > C1)) + (Src0 > C3)),
            reference=lambda in0, in1, s0, s1, imm2: (
                (in0 > s0).astype(np.float32) + (in0 > s1)
                + (in0 > np.asarray(in1, np.float32).reshape(-1, 1))
            ),
        ),
    )
    # out = (z>0) + (z>1) + (z>tau4) + (z>tau6) + u: 4 cmp + 4 add = 8 (max)
    bucket4acc = register_op(
        "BUCKET4ACC_ANT",
        Spec(
            body=(((Src0 > Zero) + (Src0 > One))
                  + ((Src0 > C0) + (Src0 > C1))) + Src1,
            reference=lambda in0, in1, s0, s1, imm2: (
                (in0 > 0).astype(np.float32) + (in0 > 1)
                + (in0 > s0) + (in0 > s1) + in1
            ),
        ),
    )
    return bucket3, bucket4acc


def _build():
    """Trace + compile the per-core Bass kernel (once per process)."""
    if _built:
        return _built[0]

    import concourse.bacc as bacc
    import concourse.mybir as mybir
    import concourse.tile as tile

    bucket3, bucket4acc = _register_dve_ops()

    f32, f16, u8 = mybir.dt.float32, mybir.dt.float16, mybir.dt.uint8
    nc = bacc.Bacc("TRN2", target_bir_lowering=False, debug=False,
                   num_devices=NCORES)

    xin_t = nc.dram_tensor("xin", [BPC, 128, SLABF], f16, kind="ExternalInput")
    wts_t = nc.dram_tensor("wts", [128, NTAPS, Cout], f16, kind="ExternalInput")
    nrm_t = nc.dram_tensor("nrm", [128, 7], f32, kind="ExternalInput")
    out_t = nc.dram_tensor("out", [BPC, 128, HALF], u8, kind="ExternalOutput")

    # pairs of chunk-pairs batched so each PE quadrant has 2 back-to-back
    # matmuls per tap (hides the 4 LDWEIGHTS under 2x448 cycles of streaming)
    BATCHES = [(0, 1), (2, 3), (4, 5), (6,)]
    GCH = 2 * NCH              # leading columns bucketized on GpSimd

    with tile.TileContext(nc) as tc:
        with (
            tc.tile_pool(name="const", bufs=1) as cpool,
            tc.tile_pool(name="slab", bufs=2) as spool,
            tc.tile_pool(name="psum", bufs=4, space="PSUM") as ppool,
            tc.tile_pool(name="z", bufs=2) as zpool,
            tc.tile_pool(name="u", bufs=2) as upool,
            tc.tile_pool(name="g", bufs=4) as gpool,
            tc.tile_pool(name="o", bufs=2) as opool,
        ):
            # first slab piece of image 0 goes out before anything else — it
            # gates the first matmul batch (rows 0-17 = cols 0-2052)
            slab0 = spool.tile([128, SLABF], f16, tag="slab")
            nc.sync.dma_start(out=slab0[:, 0:2052], in_=xin_t.ap()[0, :, 0:2052])
            wts = cpool.tile([128, NTAPS, Cout], f16)
            nc.scalar.dma_start(out=wts[:], in_=wts_t.ap())
            nrm = cpool.tile([128, 7], f32)
            nc.scalar.dma_start(out=nrm[:], in_=nrm_t.ap())

            # PE warm-up: junk matmuls over the (early-arriving) weights tile
            # while the input DMA lands, so the HAM clock gate opens
            # (1.2 -> 2.4 GHz) before the real matmuls start.  ~8 x 512 cols
            # cold ~= the 3.4us HAM window, with no memset dependency.
            wts_flat = wts[:].rearrange("p t c -> p (t c)")
            wps = ppool.tile([128, 1024], f32, name="wps", tag="ps")
            for _ in range(NWARM):
                nc.tensor.matmul(wps[:64, 0:512], wts_flat[:, 0:Cout],
                                 wts_flat[:, 0:512],
                                 tile_position=(0, 0), start=True, stop=True)

            scale, bias = nrm[:, 0:1], nrm[:, 1:2]
            tau0, tau1, tau2 = nrm[:, 2:3], nrm[:, 3:4], nrm[:, 4:5]
            tau4, tau6 = nrm[:, 5:6], nrm[:, 6:7]

            for b in range(BPC):
                # split the input DMA so early batches can start sooner;
                # finer-grained for the first image (it gates the pipeline)
                if b == 0:
                    slab = slab0
                    cuts = [2052, 4104, SLABF]
                else:
                    slab = spool.tile([128, SLABF], f16, tag="slab")
                    cuts = [0, 2242, 4484, SLABF]
                for n, (lo, hi) in enumerate(zip(cuts[:-1], cuts[1:])):
                    eng = nc.sync if n % 2 == 0 else nc.scalar
                    eng.dma_start(out=slab[:, lo:hi], in_=xin_t.ap()[b, :, lo:hi])
                slabv = slab[:].rearrange("p (r w) -> p r w", w=WPAD)

                zslab = zpool.tile([128, HALF], f16, tag="z")
                oslab = opool.tile([128, HALF], u8)
                slot0 = 0
                for batch in BATCHES:
                    # one 2-bank PSUM tile per image-half: pair batch[j] lives
                    # at columns j*512..j*512+448.  quadrant (hb, ct*64)
                    # computes chunk 2*pr+ct of image-half hb.
                    tiles = []
                    for hb in (0, 64):
                        tiles.append((ppool.tile([128, 1024], f32,
                                                 name="ps", tag="ps"), hb))
                    for t in range(NTAPS):
                        dh, dw = divmod(t, 3)
                        for j, pr in enumerate(batch):
                            for (ps, hb) in tiles:
                                for ct in range(2):
                                    chunk = 2 * pr + ct
                                    # bottom half stores padded rows 55..113:
                                    # local row index = global-55 = 4c+1+dh
                                    r0 = 4 * chunk + dh + (1 if hb else 0)
                                    nc.tensor.matmul(
                                        ps[ct * Cout:(ct + 1) * Cout,
                                           j * 512:j * 512 + NCH],
                                        wts[hb:hb + Cout, t, :],
                                        slabv[hb:hb + Cout, r0:r0 + 4,
                                              dw:dw + W],
                                        tile_position=(hb, ct * Cout),
                                        start=(t == 0), stop=(t == NTAPS - 1))
                    # drains per (batch, half, pair); zslab slots are
                    # batch-major: [T batch..., B batch...]
                    nb = len(batch)
                    for hi_, (ps, hb) in enumerate(tiles):
                        for j in range(nb):
                            dst = slot0 + hi_ * nb + j
                            nc.scalar.activation(
                                zslab[:, dst * NCH:(dst + 1) * NCH],
                                ps[:, j * 512:j * 512 + NCH],
                                mybir.ActivationFunctionType.Identity,
                                bias=bias, scale=scale)
                    slot0 += 2 * nb

                # bucketize out = sum of 7 [z > tau_k]: GpSimd chains 7
                # stock compare-accumulate ops over the leading GCH columns;
                # the custom 2-pass DVE ops cover the rest in 3 chunks
                ug = upool.tile([128, GCH], mybir.dt.bfloat16, tag="ug")
                nc.vector._custom_dve(
                    bucket3, out=ug[:], in0=zslab[:, 0:GCH],
                    in1=tau2, s0=tau0, s1=tau1)
                nc.vector._custom_dve(
                    bucket4acc, out=oslab[:, 0:GCH],
                    in0=zslab[:, 0:GCH], in1=ug[:], s0=tau4, s1=tau6)
                nc.sync.dma_start(out=out_t.ap()[b, :, 0:GCH],
                                  in_=oslab[:, 0:GCH])

                step = (HALF - GCH) // 3
                for c in range(3):
                    lo, hi = GCH + c * step, GCH + (c + 1) * step
                    u = upool.tile([128, step], mybir.dt.bfloat16, tag="u")
                    nc.vector._custom_dve(
                        bucket3, out=u[:], in0=zslab[:, lo:hi],
                        in1=tau2, s0=tau0, s1=tau1)
                    nc.vector._custom_dve(
                        bucket4acc, out=oslab[:, lo:hi],
                        in0=zslab[:, lo:hi], in1=u[:], s0=tau4, s1=tau6)
                    nc.sync.dma_start(out=out_t.ap()[b, :, lo:hi],
                                      in_=oslab[:, lo:hi])

    nc.compile()
    _built.append(nc)
    return nc


def _binarize_weights(w):
    """Exactly the reference's fp32 binarization. Returns (sign in {-1,0,1}, sw)."""
    w = np.asarray(w, np.float32)
    C = w.shape[0]
    wf = w.reshape(C, -1)
    bw = w - wf.mean(-1)[:, None, None, None]
    bw = bw / bw.reshape(C, -1).std(-1, ddof=1)[:, None, None, None]
    mean_abs = np.abs(bw).reshape(C, -1).mean(-1)
    sw = np.exp2(np.round(np.log2(mean_abs))).astype(np.float32)
    return np.sign(bw).astype(np.float32), sw


def kernel(x, w, lut):
    x = np.ascontiguousarray(np.asarray(x, np.float32))
    w = np.asarray(w, np.float32)
    lut = np.asarray(lut, np.float32)

    nc = _build()
    from concourse import bass_utils

    # ---- weights: binarize + fold the pow2 scale into the thresholds ----
    sgn, sw = _binarize_weights(w)                     # sgn [Cout,Cin,3,3]
    t64 = lut.astype(np.float64) / sw[:, None]         # [Cout,7] thresholds

    # lhsT per tap; K rows 0-63 serve PE row-group 0 (top image half),
    # rows 64-127 serve row-group 64 (bottom half) — same weights
    wts = np.empty((128, NTAPS, Cout), np.float32)
    for t in range(NTAPS):
        wts[:Cin, t, :] = sgn[:, :, t // 3, t % 3].T
    wts[Cin:] = wts[:Cin]
    wts = wts.astype(np.float16)

    # ---- normalize params: z = y*s + b with tau3 -> 0, tau5 -> 1 ----
    # s>0 always; for degenerate channels (t5 == t3) use a huge power of two
    # so [z > 1] still decides [y > t3] exactly.
    t3, t5 = t64[:, 3], t64[:, 5]
    gap = t5 - t3
    s = np.where(gap > 0, 1.0 / np.where(gap > 0, gap, 1.0), 2.0 ** 100)
    bias = -t3 * s
    taus = (t64[:, [0, 1, 2, 4, 6]] - t3[:, None]) * s[:, None]
    half = np.stack([s, bias, taus[:, 0], taus[:, 1], taus[:, 2],
                     taus[:, 3], taus[:, 4]], axis=1).astype(np.float32)
    nrm = np.empty((128, 7), np.float32)
    nrm[:Cout] = half
    nrm[Cout:] = half

    # ---- single fp16 copy, zero-padded; two overlapping horizontal halves
    # (padded rows 0-58 / 55-113) stacked in the partition dim
    xpad = np.zeros((B, Cin, HP, WPAD), np.float16)
    xpad[:, :, 1:H + 1, 1:W + 1] = x.astype(np.float16)
    xin = np.empty((B, 128, SLABF), np.float16)
    view = xin.reshape(B, 128, HROWS, WPAD)
    view[:, :Cin] = xpad[:, :, 0:HROWS]
    view[:, Cin:] = xpad[:, :, HP - HROWS:HP]

    # ---- run on the 8 cores (SPMD, batch-sharded) ----
    wts_np = np.ascontiguousarray(wts)
    nrm_np = np.ascontiguousarray(nrm)
    in_maps = [
        {
            "xin": np.ascontiguousarray(xin[c * BPC:(c + 1) * BPC]),
            "wts": wts_np,
            "nrm": nrm_np,
        }
        for c in range(NCORES)
    ]
    try:
        res = bass_utils.run_bass_kernel_spmd(nc, in_maps,
                                              core_ids=list(range(NCORES)))
    except Exception:
        # transient PJRT/compile hiccups happen occasionally; retry once
        res = bass_utils.run_bass_kernel_spmd(nc, in_maps,
                                              core_ids=list(range(NCORES)))
    global last_results
    last_results = res

    # ---- unshard: pair p<7 = top half rows 8p..8p+7, p>=7 = bottom half
    # rows 56+8(p-7)..; partitions 0-63 = first chunk (4 rows), 64-127 =
    # second chunk of the pair
    out = np.empty((B, Cout, H, W), np.float32)
    for c in range(NCORES):
        o = res.results[c]["out"]                      # [BPC, 128, HALF] u8
        # batch-major slots: per batch, T-half pairs then B-half pairs;
        # output row = 56*hb + 8*pr + 4*ct + r
        oo = o.reshape(BPC, 2, Cout, NPAIR, 4, W)      # (b, ct, ch, slot, r, w)
        slot_map = []
        for batch in ((0, 1), (2, 3), (4, 5), (6,)):
            for hb in (0, 1):
                for pr in batch:
                    slot_map.append((pr, hb))
        img = np.empty((BPC, Cout, H, W), np.uint8)
        for s, (pr, hb) in enumerate(slot_map):
            base = 56 * hb + 8 * pr
            img[:, :, base:base + 4] = oo[:, 0, :, s]
            img[:, :, base + 4:base + 8] = oo[:, 1, :, s]
        out[c * BPC:(c + 1) * BPC] = img.astype(np.float32)
    return out
